# revision 1
# baseline (speedup 1.0000x reference)
"""Distributed causal multi-head attention for 8 TRN2 NeuronCores.

Problem: B=2, T=2048, D=1024, H=16 heads (hd=64), f32 in/out.

Sharding: core i handles batch b=i//4 and head-group g=i%4 (4 heads).
Wq/Wk/Wv column-sharded ([1024, 256] per core), Wo row-sharded
([256, 1024] per core).  Each core computes a partial output projection
for its 4 heads over the full sequence; the host sums the 4 partials
per batch (the unshard step replaces the all-reduce).  As part of
sharding, the host pre-casts weights/activations to bf16 (the kernel's
compute dtype) and lays x out transposed (xT = x^T), so the device
spends no cycles on input formatting.

Per-core dataflow (matmuls bf16 on TensorEngine, f32 accumulation):
  QT,KT [256(d),2048(t)] = W^T @ x^T   (d on partitions)
  V     [2048(t),256(d)]               (t on partitions, +ones col)
  ST[k,q] = K . Q^T  -> exp (ACT, scale=1/sqrt(64)) -> PT bf16
  causal: diagonal tiles narrowed to their valid q range; only the
  128-wide diagonal block needs an affine_select mask (gpsimd)
  AV: out[q, 65] += PT[k,q]^T @ Vaug[k, 65]  (col 64 = softmax denom)
  attn[q, dv] = AV[:, :64] * recip(AV[:, 64])  (DVE per-partition)
  attnT via PE transpose -> out_partial[t,e] = attnT^T @ Wo

Emission is software-pipelined: scores of head-pair p interleave with
AV of pair p-1; the second half of the QK/V projections is injected as
PE filler into the slab-0/1 attention stream; each q-slab's epilogue
(transpose + out-proj + DMA) follows one pair behind its slab.
"""

import numpy as np
import ml_dtypes

import concourse.bass as bass
import concourse.mybir as mybir
import concourse.tile as tile
from concourse import bacc
from concourse.bass_utils import run_bass_kernel_spmd
from concourse.masks import make_identity

F32 = mybir.dt.float32
BF16 = mybir.dt.bfloat16
AF = mybir.ActivationFunctionType

T = 2048  # sequence length
D = 1024  # embed dim
NH = 4  # heads per core
HD = 64  # head dim
DH = NH * HD  # 256, sharded d per core
TT = T // 128  # 16 t tiles
DT = D // 128  # 8 embed tiles
NSLAB = 4  # q slabs of 512
SCALE = 1.0 / np.sqrt(HD)

_NC_CACHE = None

def build():
    nc = bacc.Bacc(None, target_bir_lowering=False, debug=False)

    xT_ext = nc.declare_dram_parameter("xT", [D, T], BF16, isOutput=False)
    wq = nc.declare_dram_parameter("Wq", [D, DH], BF16, isOutput=False)
    wk = nc.declare_dram_parameter("Wk", [D, DH], BF16, isOutput=False)
    wv = nc.declare_dram_parameter("Wv", [D, DH], BF16, isOutput=False)
    wo = nc.declare_dram_parameter("Wo", [DH, D], BF16, isOutput=False)
    out = nc.declare_dram_parameter("out", [T, D], F32, isOutput=True)

    with tile.TileContext(nc) as tc:
        with (
            tc.tile_pool(name="persist", bufs=1) as persist,
            tc.tile_pool(name="pt", bufs=2) as pt_pool,
            tc.tile_pool(name="opev", bufs=2) as opev_pool,
            tc.tile_pool(name="avstg", bufs=2) as avstg_pool,
            tc.tile_pool(name="recip", bufs=4) as recip_pool,
            tc.tile_pool(name="ps_st", bufs=3, space="PSUM") as ps_st,
            tc.tile_pool(name="ps_av", bufs=2, space="PSUM") as ps_av,
        ):
            def P(shape, dtype, name):
                return persist.tile(shape, dtype, name=name, tag=name)

            ident_b = P([128, 128], BF16, "ident_b")
            make_identity(nc, ident_b)

            wq_bf = P([128, DT * DH], BF16, "wq_bf")
            wk_bf = P([128, DT * DH], BF16, "wk_bf")
            wv_bf = P([128, DT * DH], BF16, "wv_bf")
            wo_bf = P([128, 2 * D], BF16, "wo_bf")
            xT = P([128, DT * T], BF16, "xT")
            QT = P([128, 2 * T], BF16, "QT")
            KT = P([128, 2 * T], BF16, "KT")
            vbuf = P([128, TT * NH * 65], BF16, "vbuf")
            attn = P([128, TT * DH], BF16, "attn")
            attnT = P([128, 2 * T], BF16, "attnT")

            # ---- input DMAs: wq first (gates first matmuls), then xT,
            # then the rest (wk needed ~3 chunks in, wv at V-proj) ----
            for dt_ in range(DT):
                eng = nc.scalar if dt_ % 2 == 0 else nc.sync
                eng.dma_start(
                    out=wq_bf[:, dt_ * DH : (dt_ + 1) * DH],
                    in_=wq[dt_ * 128 : (dt_ + 1) * 128, :],
                )
            # first wave: xT columns 0-1023 (all the prologue needs)
            for dt_ in range(DT):
                eng = nc.sync if dt_ % 2 == 0 else nc.scalar
                eng.dma_start(
                    out=xT[:, dt_ * T : dt_ * T + 1024],
                    in_=xT_ext[dt_ * 128 : (dt_ + 1) * 128, 0:1024],
                )
            for w_ext, w_bf in ((wk, wk_bf), (wv, wv_bf)):
                for dt_ in range(DT):
                    eng = nc.scalar if dt_ % 2 == 0 else nc.sync
                    eng.dma_start(
                        out=w_bf[:, dt_ * DH : (dt_ + 1) * DH],
                        in_=w_ext[dt_ * 128 : (dt_ + 1) * 128, :],
                    )
            # second wave: xT columns 1024-2047 (needed by phase-1b fillers)
            for dt_ in range(DT):
                eng = nc.sync if dt_ % 2 == 0 else nc.scalar
                eng.dma_start(
                    out=xT[:, dt_ * T + 1024 : (dt_ + 1) * T],
                    in_=xT_ext[dt_ * 128 : (dt_ + 1) * 128, 1024:2048],
                )
            for i in range(2):
                nc.scalar.dma_start(
                    out=wo_bf[:, i * D : (i + 1) * D],
                    in_=wo[i * 128 : (i + 1) * 128, :],
                )

            vb3 = vbuf.rearrange("p (t c) -> p t c", c=65)
            nc.gpsimd.memset(vb3[:, :, 64:65], 1.0)

            def qk_chunks(ch2):
                """PE-only thunks: one (w, m) QK projection chunk each."""
                thunks = []
                for w_bf, outT in ((wq_bf, QT), (wk_bf, KT)):
                    for m in range(2):
                        def go(w_bf=w_bf, outT=outT, m=m):
                            ps = ps_st.tile([128, 1024], F32, name="psst")
                            for dt_ in range(DT):
                                lhsT = w_bf[
                                    :, dt_ * DH + m * 128 : dt_ * DH + (m + 1) * 128
                                ]
                                for half in range(2):
                                    c0 = ch2 * 1024 + half * 512
                                    nc.tensor.matmul(
                                        ps[:, half * 512 : (half + 1) * 512],
                                        lhsT=lhsT,
                                        rhs=xT[:, dt_ * T + c0 : dt_ * T + c0 + 512],
                                        start=(dt_ == 0),
                                        stop=(dt_ == DT - 1),
                                    )
                            nc.vector.tensor_copy(
                                outT[:, m * T + ch2 * 1024 : m * T + (ch2 + 1) * 1024],
                                ps[:],
                            )

                        thunks.append(go)
                return thunks

            vb4 = vbuf.rearrange("p (n c) -> p n c", c=65)

            def v_chunks(tts):
                """PE-only thunks: one V-projection t-tile each."""
                thunks = []
                for tt in tts:
                    def go(tt=tt):
                        ps = ps_av.tile([128, 256], F32, name="psav", tag="psav")
                        for dt_ in range(DT):
                            nc.tensor.matmul(
                                ps[:],
                                lhsT=xT[
                                    :, dt_ * T + tt * 128 : dt_ * T + (tt + 1) * 128
                                ],
                                rhs=wv_bf[:, dt_ * DH : (dt_ + 1) * DH],
                                start=(dt_ == 0),
                                stop=(dt_ == DT - 1),
                            )
                        nc.vector.tensor_copy(
                            vb4[:, tt * NH : (tt + 1) * NH, 0:64],
                            ps.rearrange("p (n c) -> p n c", n=NH),
                        )

                    thunks.append(go)
                return thunks

            def pt_layout(s):
                """Compact per-pair PT layout: col base and q-offset per kt."""
                base, off, b = {}, {}, 0
                for kt in range(4 * (s + 1)):
                    j = kt - 4 * s
                    o = 128 * j if j > 0 else 0
                    base[kt], off[kt] = b, o
                    b += 512 - o
                return base, off

            def scores_chunks(s, h, pt):
                m, r0 = h // 2, (h % 2) * 64
                base, _ = pt_layout(s)

                def off_diag(kt):
                    def go():
                        ps = ps_st.tile([128, 1024], F32, name="psst")
                        for u in range(2):
                            nc.tensor.matmul(
                                ps[:, u * 512 : (u + 1) * 512],
                                lhsT=KT[
                                    r0 : r0 + 64,
                                    m * T + (kt + u) * 128 : m * T + (kt + u + 1) * 128,
                                ],
                                rhs=QT[
                                    r0 : r0 + 64,
                                    m * T + s * 512 : m * T + (s + 1) * 512,
                                ],
                                start=True,
                                stop=True,
                            )
                        nc.scalar.activation(
                            out=pt[:, base[kt] : base[kt] + 1024],
                            in_=ps[:],
                            func=AF.Exp,
                            scale=float(SCALE),
                        )

                    return go

                def diag2(j0):
                    # two diagonal tiles (j0, j0+1) packed into one psum/exp
                    widths = [512 - 128 * j0, 512 - 128 * (j0 + 1)]
                    wtot = sum(widths)

                    def go():
                        ps = ps_st.tile([128, 1024], F32, name="psst")
                        o = 0
                        for u, w in enumerate(widths):
                            j = j0 + u
                            kt = 4 * s + j
                            nc.tensor.matmul(
                                ps[:, o : o + w],
                                lhsT=KT[
                                    r0 : r0 + 64,
                                    m * T + kt * 128 : m * T + (kt + 1) * 128,
                                ],
                                rhs=QT[
                                    r0 : r0 + 64,
                                    m * T + s * 512 + 128 * j : m * T + (s + 1) * 512,
                                ],
                                start=True,
                                stop=True,
                            )
                            o += w
                        kt0 = 4 * s + j0
                        nc.scalar.activation(
                            out=pt[:, base[kt0] : base[kt0] + wtot],
                            in_=ps[:, 0:wtot],
                            func=AF.Exp,
                            scale=float(SCALE),
                        )
                        for u in range(2):
                            kt = 4 * s + j0 + u
                            nc.gpsimd.affine_select(
                                out=pt[:, base[kt] : base[kt] + 128],
                                in_=pt[:, base[kt] : base[kt] + 128],
                                pattern=[[1, 128]],
                                compare_op=mybir.AluOpType.is_ge,
                                fill=0.0,
                                base=0,
                                channel_multiplier=-1,
                            )

                    return go

                return [off_diag(2 * u) for u in range(2 * s)] + [diag2(0), diag2(2)]

            def av_ops(s, h, pt, split=False):
                """V-stationary AV accumulation; batched transpose+norm.
                split=True computes q-halves in separate chains so the
                first half's normalize/epilogue can overlap the second."""
                base, off = pt_layout(s)
                nk = 4 * (s + 1)
                stg = {}

                def av_go():
                    avb = ps_av.tile([128, 512], F32, name="psav", tag="psav")
                    stg["avb"] = avb
                    if not split:
                        for kt in range(nk):
                            o = off[kt]
                            nc.tensor.matmul(
                                avb[0:65, o:512],
                                lhsT=vb4[:, kt * NH + h, :],
                                rhs=pt[:, base[kt] : base[kt] + 512 - o],
                                start=(kt == 0),
                                stop=(kt == nk - 1),
                            )
                        st = avstg_pool.tile([65, 512], BF16, name="avst")
                        stg["st"] = st
                        nc.vector.tensor_copy(st[:], avb[0:65, :])
                    else:
                        # first q-half: tiles with off < 256
                        kts = [kt for kt in range(nk) if off[kt] < 256]
                        for i_, kt in enumerate(kts):
                            o = off[kt]
                            nc.tensor.matmul(
                                avb[0:65, o:256],
                                lhsT=vb4[:, kt * NH + h, :],
                                rhs=pt[:, base[kt] : base[kt] + 256 - o],
                                start=(i_ == 0),
                                stop=(i_ == len(kts) - 1),
                            )
                        st = avstg_pool.tile([65, 512], BF16, name="avst")
                        stg["st"] = st
                        nc.vector.tensor_copy(st[:, 0:256], avb[0:65, 0:256])

                def av_go2():
                    if not split:
                        return
                    avb = stg["avb"]
                    for i_, kt in enumerate(range(nk)):
                        o2 = max(off[kt] - 256, 0)
                        c0 = base[kt] + 256 - off[kt] + o2
                        nc.tensor.matmul(
                            avb[0:65, 256 + o2 : 512],
                            lhsT=vb4[:, kt * NH + h, :],
                            rhs=pt[:, c0 : c0 + 256 - o2],
                            start=(kt == 0),
                            stop=(kt == nk - 1),
                        )
                    st = stg["st"]
                    nc.vector.tensor_copy(st[:, 256:512], stg["avb"][0:65, 256:512])

                pnst = {}

                def tr_go(qi):
                    def go():
                        st = stg["st"]
                        if qi == 0:
                            pnst["pn"] = ps_av.tile(
                                [128, 264], BF16, name="psn", tag="psav"
                            )
                            pnst["rc"] = recip_pool.tile(
                                [128, 4], F32, name="rc"
                            )
                        pn = pnst["pn"]
                        nc.tensor.transpose(
                            pn[:, qi * 66 : qi * 66 + 65],
                            st[:, qi * 128 : (qi + 1) * 128],
                            ident_b[0:65, 0:65],
                        )
                        last = (qi == 1) if split else (qi == 3)
                        if last or qi == 3:
                            lo = 0 if (split and qi == 1) else (2 if split else 0)
                            rc = pnst["rc"]
                            nc.vector.reciprocal(
                                rc[:, lo : lo + 2] if split else rc[:],
                                pn.rearrange("p (n c) -> p n c", c=66)[
                                    :, lo : lo + 2 if split else 4, 64
                                ]
                                if split
                                else pn.rearrange("p (n c) -> p n c", c=66)[:, :, 64],
                            )

                    return go

                def norm_go(qi):
                    def go():
                        qt = 4 * s + qi
                        pn, rc = pnst["pn"], pnst["rc"]
                        nc.vector.tensor_scalar_mul(
                            attn[:, qt * DH + h * 64 : qt * DH + (h + 1) * 64],
                            pn[:, qi * 66 : qi * 66 + 64],
                            rc[:, qi : qi + 1],
                        )

                    return go

                if not split:
                    return (
                        [av_go]
                        + [tr_go(qi) for qi in range(4)]
                        + [norm_go(qi) for qi in range(4)]
                    )
                return [
                    av_go,
                    tr_go(0), tr_go(1), norm_go(0), norm_go(1),
                    av_go2,
                    tr_go(2), tr_go(3), norm_go(2), norm_go(3),
                ]

            at3 = attnT.rearrange("p (i t) -> p i t", i=2)

            def epilogue_ops(s):
                ops = []
                for qt in range(4 * s, 4 * (s + 1)):
                    def tr(qt=qt):
                        ps = ps_av.tile([128, 256], BF16, name="pstrb", tag="psav")
                        for i in range(2):
                            nc.tensor.transpose(
                                ps[:, i * 128 : (i + 1) * 128],
                                attn[:, qt * DH + i * 128 : qt * DH + (i + 1) * 128],
                                ident_b[:],
                            )
                        nc.vector.tensor_copy(
                            at3[:, :, qt * 128 : (qt + 1) * 128],
                            ps.rearrange("p (i c) -> p i c", i=2),
                        )

                    ops.append(tr)
                for tt in range(4 * s, 4 * (s + 1)):
                    def op_(tt=tt):
                        ps = ps_st.tile([128, 1024], F32, name="psst")
                        for i in range(2):
                            lhsT = attnT[:, i * T + tt * 128 : i * T + (tt + 1) * 128]
                            for ec in range(2):
                                nc.tensor.matmul(
                                    ps[:, ec * 512 : (ec + 1) * 512],
                                    lhsT=lhsT,
                                    rhs=wo_bf[
                                        :, i * D + ec * 512 : i * D + (ec + 1) * 512
                                    ],
                                    start=(i == 0),
                                    stop=(i == 1),
                                )
                        ev = opev_pool.tile([128, 1024], F32, name="ev")
                        for ec in range(2):
                            nc.vector.tensor_copy(
                                ev[:, ec * 512 : (ec + 1) * 512],
                                ps[:, ec * 512 : (ec + 1) * 512],
                            )
                        nc.sync.dma_start(
                            out=out[tt * 128 : (tt + 1) * 128, :], in_=ev[:]
                        )

                    ops.append(op_)
                return ops

            def emit_slab_epilogue(s):
                for qt in range(4 * s, 4 * (s + 1)):
                    ps = ps_av.tile([128, 256], BF16, name="pstrb", tag="psav")
                    for i in range(2):
                        nc.tensor.transpose(
                            ps[:, i * 128 : (i + 1) * 128],
                            attn[:, qt * DH + i * 128 : qt * DH + (i + 1) * 128],
                            ident_b[:],
                        )
                    nc.vector.tensor_copy(
                        at3[:, :, qt * 128 : (qt + 1) * 128],
                        ps.rearrange("p (i c) -> p i c", i=2),
                    )
                for tt in range(4 * s, 4 * (s + 1)):
                    ps = ps_st.tile([128, 1024], F32, name="psst")
                    for i in range(2):
                        lhsT = attnT[:, i * T + tt * 128 : i * T + (tt + 1) * 128]
                        for ec in range(2):
                            nc.tensor.matmul(
                                ps[:, ec * 512 : (ec + 1) * 512],
                                lhsT=lhsT,
                                rhs=wo_bf[:, i * D + ec * 512 : i * D + (ec + 1) * 512],
                                start=(i == 0),
                                stop=(i == 1),
                            )
                    ev = opev_pool.tile([128, 1024], F32, name="ev")
                    for ec in range(2):
                        nc.vector.tensor_copy(
                            ev[:, ec * 512 : (ec + 1) * 512],
                            ps[:, ec * 512 : (ec + 1) * 512],
                        )
                    nc.sync.dma_start(
                        out=out[tt * 128 : (tt + 1) * 128, :], in_=ev[:]
                    )

            def interleave(a, b):
                if not a:
                    return list(b)
                if not b:
                    return list(a)
                res = []
                nb, na, bi = len(b), len(a), 0
                for i, op in enumerate(a):
                    res.append(op)
                    want = (i + 1) * nb // na
                    while bi < want:
                        res.append(b[bi])
                        bi += 1
                res.extend(b[bi:])
                return res

            # ---- minimal prologue: first halves of projections ----
            for op in qk_chunks(0):
                op()
            for op in v_chunks(range(0, 8)):
                op()

            # remaining projection work, injected as PE filler into the
            # slab-0/1 attention stream
            fillers = qk_chunks(1) + v_chunks(range(8, 16))
            f_per_idx = [2, 2, 2, 2, 1, 1, 1, 1]  # idx 0..7 -> 12 fillers

            pairs = [(s, h) for s in range(NSLAB) for h in range(NH)]
            pts = {}
            prev = None
            fi = 0
            for idx in range(len(pairs) + 1):
                sc = []
                if idx < len(pairs):
                    s, h = pairs[idx]
                    pts[idx] = pt_pool.tile([128, TT * 512], BF16, name="pt")
                    sc = scores_chunks(s, h, pts[idx])
                av = []
                if prev is not None:
                    ps_, ph_ = pairs[prev]
                    av = av_ops(ps_, ph_, pts[prev], split=(prev == len(pairs) - 1))
                fill = []
                if idx < len(f_per_idx):
                    n = f_per_idx[idx]
                    fill = fillers[fi : fi + n]
                    fi += n
                epi = []
                if prev is not None:
                    dss, dhh = pairs[prev]
                    if dhh == 0 and dss >= 1:
                        epi = epilogue_ops(dss - 1)
                if idx == len(pairs):
                    # final iteration: interleave last-slab epilogue with
                    # the split AV halves of the last pair
                    eops = epilogue_ops(NSLAB - 1)
                    for op in av[0:5]:
                        op()
                    eops[0]()  # tr qt12
                    eops[1]()  # tr qt13
                    eops[4]()  # outproj tt12
                    for op in av[5:]:
                        op()
                    eops[5]()  # outproj tt13
                    for k in (2, 3, 6, 7):
                        eops[k]()
                else:
                    for op in interleave(sc, av + fill + epi):
                        op()
                prev = idx

    nc.compile()
    return nc


def _get_nc():
    global _NC_CACHE
    if _NC_CACHE is None:
        _NC_CACHE = build()
    return _NC_CACHE


def make_in_maps(x, Wq, Wk, Wv, Wo):
    bf = ml_dtypes.bfloat16
    x = np.asarray(x, dtype=np.float32)
    WqT = np.asarray(Wq, dtype=np.float32).astype(bf)
    WkT = np.asarray(Wk, dtype=np.float32).astype(bf)
    WvT = np.asarray(Wv, dtype=np.float32).astype(bf)
    WoT = np.asarray(Wo, dtype=np.float32).astype(bf)
    xTb = [np.ascontiguousarray(x[b].T.astype(bf)) for b in range(2)]
    in_maps = []
    for core in range(8):
        b, g = core // 4, core % 4
        sl = slice(g * DH, (g + 1) * DH)
        in_maps.append(
            {
                "xT": xTb[b],
                "Wq": np.ascontiguousarray(WqT[:, sl]),
                "Wk": np.ascontiguousarray(WkT[:, sl]),
                "Wv": np.ascontiguousarray(WvT[:, sl]),
                "Wo": np.ascontiguousarray(WoT[sl, :]),
            }
        )
    return in_maps


def unshard(results):
    out = np.empty((2, T, D), np.float32)
    for b in range(2):
        out[b] = results[4 * b]["out"]
        for g in range(1, 4):
            out[b] += results[4 * b + g]["out"]
    return out


def kernel(x, Wq, Wk, Wv, Wo):
    nc = _get_nc()
    in_maps = make_in_maps(x, Wq, Wk, Wv, Wo)
    res = run_bass_kernel_spmd(nc, in_maps, core_ids=list(range(8)))
    return unshard(res.results)



# revision 5
# speedup vs baseline: 1.0129x; 1.0129x over previous
"""Distributed causal multi-head attention for 8 TRN2 NeuronCores (v2).

Problem: B=2, T=2048, D=1024, H=16 heads (hd=64), f32 in/out.

Sharding: core i handles batch b=i//4 and head-group g=i%4 (4 heads).
Wq/Wk/Wv column-sharded ([1024, 256] per core), Wo row-sharded
([256, 1024] per core).  Each core computes a partial output projection
for its 4 heads over the full sequence; the host sums the 4 partials
per batch (the unshard step replaces the all-reduce).  Weights and
activations are pre-cast to bf16 on the host; x is laid out transposed
(xT = x^T).  Output partials are shipped bf16 and summed f32 on host.

v2 dataflow changes vs v1:
  - softmax normalize applied directly on the AV output (O^T layout,
    dh on partitions) via: den-row cast -> K=1 ones-matmul broadcast
    (den replicated over 64 partitions in PSUM) -> reciprocal_approx
    -> one tensor_tensor multiply writing attnT in place.  This deletes
    all 96 PE transposes of v1 (4 per pair + 2 per q-tile).
  - odd heads (attnT partitions 64-127) land via a cheap SBUF->SBUF
    DMA partition shift (DVE lanes are partition-locked).
  - QKV projections stream as 512-column wavefront thunks used as PE
    filler: wave c feeds q-slab c, emitted one slab ahead, so attention
    starts ~3us into the kernel and the PE never starves on input DMA.
  - out projection results are cast bf16 and DMA'd per q-tile (halves
    output traffic; host sums partials in f32).
  - exp table-load is prefetched with a dummy activation at t=0.
"""

import numpy as np
import ml_dtypes

import concourse.bass as bass
import concourse.mybir as mybir
import concourse.tile as tile
from concourse import bacc
from concourse.bass_utils import run_bass_kernel_spmd

F32 = mybir.dt.float32
BF16 = mybir.dt.bfloat16
AF = mybir.ActivationFunctionType
MULT = mybir.AluOpType.mult

T = 2048  # sequence length
D = 1024  # embed dim
NH = 4  # heads per core
HD = 64  # head dim
DH = NH * HD  # 256, sharded d per core
TT = T // 128  # 16 t tiles
DT = D // 128  # 8 embed tiles
NSLAB = 4  # q slabs of 512
SCALE = 1.0 / np.sqrt(HD)

_NC_CACHE = None


def build():
    nc = bacc.Bacc(None, target_bir_lowering=False, debug=False)

    xT_ext = nc.declare_dram_parameter("xT", [D, T], BF16, isOutput=False)
    wq = nc.declare_dram_parameter("Wq", [D, DH], BF16, isOutput=False)
    wk = nc.declare_dram_parameter("Wk", [D, DH], BF16, isOutput=False)
    wv = nc.declare_dram_parameter("Wv", [D, DH], BF16, isOutput=False)
    wo = nc.declare_dram_parameter("Wo", [DH, D], BF16, isOutput=False)
    out = nc.declare_dram_parameter("out", [T, D], BF16, isOutput=True)

    with tile.TileContext(nc) as tc:
        with (
            tc.tile_pool(name="persist", bufs=1) as persist,
            tc.tile_pool(name="pt", bufs=2) as pt_pool,
            tc.tile_pool(name="den", bufs=2) as den_pool,
            tc.tile_pool(name="rc", bufs=2) as rc_pool,
            tc.tile_pool(name="stg", bufs=2) as stg_pool,
            tc.tile_pool(name="osb", bufs=2) as osb_pool,
            tc.tile_pool(name="ps_big", bufs=2, space="PSUM") as ps_big,
            tc.tile_pool(name="ps_fill", bufs=2, space="PSUM") as ps_fill,
            tc.tile_pool(name="ps_av", bufs=2, space="PSUM") as ps_av,
        ):
            def P(shape, dtype, name):
                return persist.tile(shape, dtype, name=name, tag=name)

            ones_b = P([128, 64], BF16, "ones_b")
            nc.gpsimd.memset(ones_b[:], 1.0)
            # exp table prefetch: overlaps the ~2.7us ACT_TABLE_LOAD with
            # the input DMAs instead of paying it at the first real score
            junk = P([128, 16], F32, "junk")
            nc.gpsimd.memset(junk[:], 0.0)
            jout = P([128, 16], F32, "jout")
            nc.scalar.activation(out=jout[:], in_=junk[:], func=AF.Exp, scale=1.0)

            wq_bf = P([128, DT * DH], BF16, "wq_bf")
            wk_bf = P([128, DT * DH], BF16, "wk_bf")
            wv_bf = P([128, DT * DH], BF16, "wv_bf")
            wo_bf = P([128, 2 * D], BF16, "wo_bf")
            xT = P([128, DT * T], BF16, "xT")
            QT = P([128, 2 * T], BF16, "QT")
            KT = P([128, 2 * T], BF16, "KT")
            vbuf = P([128, TT * NH * 65], BF16, "vbuf")
            attnT = P([128, 2 * T], BF16, "attnT")

            # ---- input DMAs: one descriptor per tensor / xT 512-chunk.
            # gpsimd+sync queues (scalar stays free for the exp stream) ----
            def w_in(w_ext):
                return w_ext.rearrange("(dt p) c -> p dt c", p=128)

            xT3 = xT.rearrange("p (dt t) -> p dt t", dt=DT)
            xe3 = xT_ext.rearrange("(dt p) t -> p dt t", p=128)

            def w3(w_bf):
                return w_bf.rearrange("p (dt c) -> p dt c", dt=DT)

            nc.gpsimd.dma_start(out=w3(wq_bf)[:], in_=w_in(wq))
            nc.sync.dma_start(out=xT3[:, :, 0:512], in_=xe3[:, :, 0:512])
            nc.gpsimd.dma_start(out=w3(wv_bf)[:], in_=w_in(wv))
            nc.sync.dma_start(out=w3(wk_bf)[:], in_=w_in(wk))
            nc.gpsimd.dma_start(out=xT3[:, :, 512:1024], in_=xe3[:, :, 512:1024])
            nc.sync.dma_start(
                out=wo_bf.rearrange("p (i c) -> p i c", i=2)[:],
                in_=wo.rearrange("(i p) c -> p i c", p=128),
            )
            nc.sync.dma_start(out=xT3[:, :, 1024:1536], in_=xe3[:, :, 1024:1536])
            nc.gpsimd.dma_start(out=xT3[:, :, 1536:2048], in_=xe3[:, :, 1536:2048])

            vb3 = vbuf.rearrange("p (t c) -> p t c", c=65)
            nc.gpsimd.memset(vb3[:, :, 64:65], 1.0)
            vb4 = vbuf.rearrange("p (n c) -> p n c", c=65)

            # ---- projection wavefront thunks (PE filler) ----
            def qk_thunks(c):
                th = []
                for w_bf, outT in ((wq_bf, QT), (wk_bf, KT)):
                    for m in range(2):
                        def go(w_bf=w_bf, outT=outT, m=m, c=c):
                            ps = ps_fill.tile([128, 512], F32, name="qk", tag="fill")
                            for dt_ in range(DT):
                                nc.tensor.matmul(
                                    ps[:],
                                    lhsT=w_bf[
                                        :, dt_ * DH + m * 128 : dt_ * DH + (m + 1) * 128
                                    ],
                                    rhs=xT[
                                        :, dt_ * T + c * 512 : dt_ * T + (c + 1) * 512
                                    ],
                                    start=(dt_ == 0),
                                    stop=(dt_ == DT - 1),
                                )
                            nc.vector.tensor_copy(
                                outT[:, m * T + c * 512 : m * T + (c + 1) * 512],
                                ps[:],
                            )

                        th.append(go)
                return th

            def v_thunks(tts):
                th = []
                for tt in tts:
                    def go(tt=tt):
                        ps = ps_fill.tile([128, 256], F32, name="vp", tag="fill")
                        for dt_ in range(DT):
                            nc.tensor.matmul(
                                ps[:],
                                lhsT=xT[
                                    :, dt_ * T + tt * 128 : dt_ * T + (tt + 1) * 128
                                ],
                                rhs=wv_bf[:, dt_ * DH : (dt_ + 1) * DH],
                                start=(dt_ == 0),
                                stop=(dt_ == DT - 1),
                            )
                        nc.vector.tensor_copy(
                            vb4[:, tt * NH : (tt + 1) * NH, 0:64],
                            ps.rearrange("p (n c) -> p n c", n=NH),
                        )

                    th.append(go)
                return th

            # ---- scores ----
            def pt_layout(s):
                """Compact per-pair PT layout: col base and q-offset per kt."""
                base, off, b = {}, {}, 0
                for kt in range(4 * (s + 1)):
                    j = kt - 4 * s
                    o = 128 * j if j > 0 else 0
                    base[kt], off[kt] = b, o
                    b += 512 - o
                return base, off

            def scores_chunks(s, h, pt):
                m, r0 = h // 2, (h % 2) * 64
                base, _ = pt_layout(s)

                def off_diag(kt):
                    def go():
                        ps = ps_big.tile([128, 1024], F32, name="psst")
                        for u in range(2):
                            nc.tensor.matmul(
                                ps[:, u * 512 : (u + 1) * 512],
                                lhsT=KT[
                                    r0 : r0 + 64,
                                    m * T + (kt + u) * 128 : m * T + (kt + u + 1) * 128,
                                ],
                                rhs=QT[
                                    r0 : r0 + 64,
                                    m * T + s * 512 : m * T + (s + 1) * 512,
                                ],
                                start=True,
                                stop=True,
                            )
                        nc.scalar.activation(
                            out=pt[:, base[kt] : base[kt] + 1024],
                            in_=ps[:],
                            func=AF.Exp,
                            scale=float(SCALE),
                        )

                    return go

                def diag2(j0):
                    widths = [512 - 128 * j0, 512 - 128 * (j0 + 1)]
                    wtot = sum(widths)

                    def go():
                        ps = ps_big.tile([128, 1024], F32, name="psst")
                        o = 0
                        for u, w in enumerate(widths):
                            j = j0 + u
                            kt = 4 * s + j
                            nc.tensor.matmul(
                                ps[:, o : o + w],
                                lhsT=KT[
                                    r0 : r0 + 64,
                                    m * T + kt * 128 : m * T + (kt + 1) * 128,
                                ],
                                rhs=QT[
                                    r0 : r0 + 64,
                                    m * T + s * 512 + 128 * j : m * T + (s + 1) * 512,
                                ],
                                start=True,
                                stop=True,
                            )
                            o += w
                        kt0 = 4 * s + j0
                        nc.scalar.activation(
                            out=pt[:, base[kt0] : base[kt0] + wtot],
                            in_=ps[:, 0:wtot],
                            func=AF.Exp,
                            scale=float(SCALE),
                        )
                        for u in range(2):
                            kt = 4 * s + j0 + u
                            nc.gpsimd.affine_select(
                                out=pt[:, base[kt] : base[kt] + 128],
                                in_=pt[:, base[kt] : base[kt] + 128],
                                pattern=[[1, 128]],
                                compare_op=mybir.AluOpType.is_ge,
                                fill=0.0,
                                base=0,
                                channel_multiplier=-1,
                            )

                    return go

                return [off_diag(2 * u) for u in range(2 * s)] + [diag2(0), diag2(2)]

            # ---- AV + normalize ----
            def av_ops_ab(s, h, pt, ref):
                """AV accumulation split: A = off-diag k-tiles (exp long
                done), B = the 4 diagonal k-tiles (gated on last exps)."""
                base, off = pt_layout(s)
                nk = 4 * (s + 1)

                def av_a():
                    avb = ps_av.tile([128, 512], F32, name="psav", tag="psav")
                    ref["avb"] = avb
                    for kt in range(4 * s):
                        nc.tensor.matmul(
                            avb[0:65, 0:512],
                            lhsT=vb4[:, kt * NH + h, :],
                            rhs=pt[:, base[kt] : base[kt] + 512],
                            start=(kt == 0),
                            stop=False,
                        )

                def av_b():
                    avb = ref["avb"]
                    for kt in range(4 * s, nk):
                        o = off[kt]
                        nc.tensor.matmul(
                            avb[0:65, o:512],
                            lhsT=vb4[:, kt * NH + h, :],
                            rhs=pt[:, base[kt] : base[kt] + 512 - o],
                            start=(kt == 0),
                            stop=(kt == nk - 1),
                        )

                return av_a, av_b

            def norm_ops(s, h, ref, q0, q1):
                """den row -> bf16 -> K=1 ones-matmul broadcast -> recip
                -> tensor_tensor multiply -> attnT (DMA shift for odd h)."""
                i_c = h // 2
                c0 = i_c * T + s * 512
                odd = h % 2 == 1
                st = {}

                def d1():
                    den = den_pool.tile([128, 512], BF16, name="den")
                    st["den"] = den
                    nc.vector.tensor_copy(
                        den[64:65, q0:q1], ref["avb"][64:65, q0:q1]
                    )

                def m1():
                    denb = ps_fill.tile([128, 512], F32, name="denb", tag="fill")
                    st["denb"] = denb
                    nc.tensor.matmul(
                        denb[0:64, q0:q1],
                        lhsT=ones_b[64:65, 0:64],
                        rhs=st["den"][64:65, q0:q1],
                        start=True,
                        stop=True,
                    )

                def d2():
                    rc = rc_pool.tile([128, 512], F32, name="rc")
                    st["rc"] = rc
                    nc.vector.reciprocal_approx_fast(
                        rc[0:64, q0:q1], st["denb"][0:64, q0:q1]
                    )

                def d3():
                    if odd:
                        stg = stg_pool.tile([128, 512], BF16, name="stg")
                        st["stg"] = stg
                        dst = stg[0:64, q0:q1]
                    else:
                        dst = attnT[0:64, c0 + q0 : c0 + q1]
                    nc.vector.tensor_tensor(
                        out=dst,
                        in0=ref["avb"][0:64, q0:q1],
                        in1=st["rc"][0:64, q0:q1],
                        op=MULT,
                    )

                def d4():
                    nc.gpsimd.dma_start(
                        out=attnT[64:128, c0 + q0 : c0 + q1],
                        in_=st["stg"][0:64, q0:q1],
                    )

                ops = [d1, m1, d2, d3]
                if odd:
                    ops.append(d4)
                return ops

            # ---- out projection epilogue ----
            def epilogue_ops(s):
                ops = []
                for tt in range(4 * s, 4 * (s + 1)):
                    st = {}

                    def op_ec(ec, tt=tt, st=st):
                        def go():
                            ps = ps_fill.tile([128, 512], F32, name="opj", tag="fill")
                            st[ec] = ps
                            for i in range(2):
                                nc.tensor.matmul(
                                    ps[:],
                                    lhsT=attnT[
                                        :, i * T + tt * 128 : i * T + (tt + 1) * 128
                                    ],
                                    rhs=wo_bf[
                                        :, i * D + ec * 512 : i * D + (ec + 1) * 512
                                    ],
                                    start=(i == 0),
                                    stop=(i == 1),
                                )

                        return go

                    def cast_dma(ec, tt=tt, st=st):
                        def go():
                            if ec == 0:
                                st["osb"] = osb_pool.tile(
                                    [128, 1024], BF16, name="osb"
                                )
                            nc.vector.tensor_copy(
                                st["osb"][:, ec * 512 : (ec + 1) * 512], st[ec][:]
                            )
                            if ec == 1:
                                nc.sync.dma_start(
                                    out=out[tt * 128 : (tt + 1) * 128, :],
                                    in_=st["osb"][:],
                                )

                        return go

                    ops.extend([op_ec(0), op_ec(1), cast_dma(0), cast_dma(1)])
                return ops

            def interleave(a, b):
                if not a:
                    return list(b)
                if not b:
                    return list(a)
                res = []
                nb, na, bi = len(b), len(a), 0
                for i, op in enumerate(a):
                    res.append(op)
                    want = (i + 1) * nb // na
                    while bi < want:
                        res.append(b[bi])
                        bi += 1
                res.extend(b[bi:])
                return res

            # ---- prologue: wave 0 (QK chunk 0 + V tiles 0-3) ----
            for op in qk_thunks(0):
                op()
            for op in v_thunks(range(0, 4)):
                op()

            waves = {
                0: qk_thunks(1) + v_thunks(range(4, 8)),
                1: qk_thunks(2) + v_thunks(range(8, 12)),
                2: qk_thunks(3) + v_thunks(range(12, 16)),
            }

            pairs = [(s, h) for s in range(NSLAB) for h in range(NH)]
            pts = {}
            refs = {}
            prev = None
            for idx in range(len(pairs)):
                s, h = pairs[idx]
                pts[idx] = pt_pool.tile([128, TT * 512], BF16, name="pt")
                sc = scores_chunks(s, h, pts[idx])
                blist = []
                if prev is not None:
                    ps_, ph_ = pairs[prev]
                    refs[prev] = {}
                    av_a, av_b = av_ops_ab(ps_, ph_, pts[prev], refs[prev])
                    nrm = norm_ops(ps_, ph_, refs[prev], 0, 512)
                    epi = []
                    if ph_ == 0 and ps_ >= 1:
                        epi = epilogue_ops(ps_ - 1)
                    fill = waves[s][2 * (idx % 4) : 2 * (idx % 4) + 2] if s < 3 else []
                    blist = (
                        [av_a]
                        + epi[:8]
                        + fill
                        + [av_b]
                        + nrm
                        + epi[8:]
                    )
                else:
                    fill = waves[0][0:2]
                    blist = list(fill)
                for op in interleave(sc, blist):
                    op()
                prev = idx

            # ---- final pair (3,3): split AV/norm halves interleaved with
            # the slab-3 epilogue ----
            s_, h_ = 3, 3
            ref = {}
            base, off = pt_layout(s_)
            nk = 16
            pt = pts[15]

            def av15a():
                avb = ps_av.tile([128, 512], F32, name="psav", tag="psav")
                ref["avb"] = avb
                kts = [kt for kt in range(nk) if off[kt] < 256]
                for i_, kt in enumerate(kts):
                    o = off[kt]
                    nc.tensor.matmul(
                        avb[0:65, o:256],
                        lhsT=vb4[:, kt * NH + h_, :],
                        rhs=pt[:, base[kt] : base[kt] + 256 - o],
                        start=(i_ == 0),
                        stop=(i_ == len(kts) - 1),
                    )

            def av15b():
                avb = ref["avb"]
                for kt in range(nk):
                    o2 = max(off[kt] - 256, 0)
                    c0 = base[kt] + 256 - off[kt] + o2
                    nc.tensor.matmul(
                        avb[0:65, 256 + o2 : 512],
                        lhsT=vb4[:, kt * NH + h_, :],
                        rhs=pt[:, c0 : c0 + 256 - o2],
                        start=(kt == 0),
                        stop=(kt == nk - 1),
                    )

            nrm_a = norm_ops(s_, h_, ref, 0, 256)
            nrm_b = norm_ops(s_, h_, ref, 256, 512)
            eops = epilogue_ops(3)

            av15a()
            for op_ in nrm_a:
                op_()
            for op_ in eops[0:4]:  # tt12
                op_()
            av15b()
            for op_ in eops[4:8]:  # tt13
                op_()
            for op_ in nrm_b:
                op_()
            for op_ in eops[8:16]:  # tt14, tt15
                op_()

    nc.compile()
    return nc


def _get_nc():
    global _NC_CACHE
    if _NC_CACHE is None:
        _NC_CACHE = build()
    return _NC_CACHE


def make_in_maps(x, Wq, Wk, Wv, Wo):
    bf = ml_dtypes.bfloat16
    x = np.asarray(x, dtype=np.float32)
    WqT = np.asarray(Wq, dtype=np.float32).astype(bf)
    WkT = np.asarray(Wk, dtype=np.float32).astype(bf)
    WvT = np.asarray(Wv, dtype=np.float32).astype(bf)
    WoT = np.asarray(Wo, dtype=np.float32).astype(bf)
    xTb = [np.ascontiguousarray(x[b].T.astype(bf)) for b in range(2)]
    in_maps = []
    for core in range(8):
        b, g = core // 4, core % 4
        sl = slice(g * DH, (g + 1) * DH)
        in_maps.append(
            {
                "xT": xTb[b],
                "Wq": np.ascontiguousarray(WqT[:, sl]),
                "Wk": np.ascontiguousarray(WkT[:, sl]),
                "Wv": np.ascontiguousarray(WvT[:, sl]),
                "Wo": np.ascontiguousarray(WoT[sl, :]),
            }
        )
    return in_maps


def unshard(results):
    out = np.empty((2, T, D), np.float32)
    for b in range(2):
        acc = results[4 * b]["out"].astype(np.float32)
        for g in range(1, 4):
            acc = acc + results[4 * b + g]["out"].astype(np.float32)
        out[b] = acc
    return out


def kernel(x, Wq, Wk, Wv, Wo):
    nc = _get_nc()
    in_maps = make_in_maps(x, Wq, Wk, Wv, Wo)
    res = run_bass_kernel_spmd(nc, in_maps, core_ids=list(range(8)))
    return unshard(res.results)


# revision 9
# speedup vs baseline: 1.0263x; 1.0132x over previous
"""Distributed causal multi-head attention for 8 TRN2 NeuronCores (v2).

Problem: B=2, T=2048, D=1024, H=16 heads (hd=64), f32 in/out.

Sharding: core i handles batch b=i//4 and head-group g=i%4 (4 heads).
Wq/Wk/Wv column-sharded ([1024, 256] per core), Wo row-sharded
([256, 1024] per core).  Each core computes a partial output projection
for its 4 heads over the full sequence; the host sums the 4 partials
per batch (the unshard step replaces the all-reduce).  Weights and
activations are pre-cast to bf16 on the host; x is laid out transposed
(xT = x^T).  Output partials are shipped bf16 and summed f32 on host.

v2 dataflow changes vs v1:
  - softmax normalize applied directly on the AV output (O^T layout,
    dh on partitions) via: den-row cast -> K=1 ones-matmul broadcast
    (den replicated over 64 partitions in PSUM) -> reciprocal_approx
    -> one tensor_tensor multiply writing attnT in place.  This deletes
    all 96 PE transposes of v1 (4 per pair + 2 per q-tile).
  - odd heads (attnT partitions 64-127) land via a cheap SBUF->SBUF
    DMA partition shift (DVE lanes are partition-locked).
  - QKV projections stream as 512-column wavefront thunks used as PE
    filler: wave c feeds q-slab c, emitted one slab ahead, so attention
    starts ~3us into the kernel and the PE never starves on input DMA.
  - out projection results are cast bf16 and DMA'd per q-tile (halves
    output traffic; host sums partials in f32).
  - exp table-load is prefetched with a dummy activation at t=0.
"""

import numpy as np
import ml_dtypes

import concourse.bass as bass
import concourse.mybir as mybir
import concourse.tile as tile
from concourse import bacc
from concourse.bass_utils import run_bass_kernel_spmd

F32 = mybir.dt.float32
BF16 = mybir.dt.bfloat16
AF = mybir.ActivationFunctionType
MULT = mybir.AluOpType.mult

T = 2048  # sequence length
D = 1024  # embed dim
NH = 4  # heads per core
HD = 64  # head dim
DH = NH * HD  # 256, sharded d per core
TT = T // 128  # 16 t tiles
DT = D // 128  # 8 embed tiles
NSLAB = 4  # q slabs of 512
SCALE = 1.0 / np.sqrt(HD)

_NC_CACHE = None


def build():
    nc = bacc.Bacc(None, target_bir_lowering=False, debug=False)

    # inputs are shipped as ready-to-DMA SBUF images (see make_in_maps):
    # xT_img[p, c*4096 + dt*512 + j] = x[c*512+j, dt*128+p]  (chunk-major)
    # wq/wk m-major [p, m*1024 + dt*128 + c]; wv dt-major [p, dt*256 + c];
    # wo i-major [p, i*1024 + c]
    xT_img = nc.declare_dram_parameter("xT", [128, NSLAB * DT * 512], BF16, isOutput=False)
    wq = nc.declare_dram_parameter("Wq", [128, 2 * DT * 128], BF16, isOutput=False)
    wk = nc.declare_dram_parameter("Wk", [128, 2 * DT * 128], BF16, isOutput=False)
    wv = nc.declare_dram_parameter("Wv", [128, DT * DH], BF16, isOutput=False)
    wo = nc.declare_dram_parameter("Wo", [128, 2 * D], BF16, isOutput=False)
    out = nc.declare_dram_parameter("out", [T, D], BF16, isOutput=True)

    with tile.TileContext(nc) as tc:
        with (
            tc.tile_pool(name="persist", bufs=1) as persist,
            tc.tile_pool(name="pt", bufs=2) as pt_pool,
            tc.tile_pool(name="den", bufs=2) as den_pool,
            tc.tile_pool(name="rc", bufs=2) as rc_pool,
            tc.tile_pool(name="stg", bufs=2) as stg_pool,
            tc.tile_pool(name="osb", bufs=2) as osb_pool,
            tc.tile_pool(name="ps_big", bufs=2, space="PSUM") as ps_big,
            tc.tile_pool(name="ps_fill", bufs=2, space="PSUM") as ps_fill,
            tc.tile_pool(name="ps_av", bufs=2, space="PSUM") as ps_av,
        ):
            def P(shape, dtype, name):
                return persist.tile(shape, dtype, name=name, tag=name)

            ones_b = P([128, 64], BF16, "ones_b")
            junk = P([128, 16], F32, "junk")
            jout = P([128, 16], F32, "jout")

            wq_bf = P([128, DT * DH], BF16, "wq_bf")
            wk_bf = P([128, DT * DH], BF16, "wk_bf")
            wv_bf = P([128, DT * DH], BF16, "wv_bf")
            wo_bf = P([128, 2 * D], BF16, "wo_bf")
            xT = P([128, DT * T], BF16, "xT")
            QT = P([128, 2 * T], BF16, "QT")
            KT = P([128, 2 * T], BF16, "KT")
            vbuf = P([128, TT * NH * 65], BF16, "vbuf")
            attnT = P([128, 2 * T], BF16, "attnT")

            # ---- input DMAs: issued first, pre-arranged images, 8-32KB
            # contiguous lines, split across sync/gpsimd/vector queues ----
            xT3 = xT.rearrange("p (dt t) -> p dt t", dt=DT)

            def xi3(c, d0, d1):
                return xT_img[:, c * 4096 + d0 * 512 : c * 4096 + d1 * 512].rearrange(
                    "p (dt t) -> p dt t", dt=d1 - d0
                )

            def xc_dma(eng, c, d0, d1):
                eng.dma_start(
                    out=xT3[:, d0:d1, c * 512 : (c + 1) * 512], in_=xi3(c, d0, d1)
                )

            nc.sync.dma_start(out=wq_bf[:, 0:1024], in_=wq[:, 0:1024])
            nc.gpsimd.dma_start(out=wq_bf[:, 1024:2048], in_=wq[:, 1024:2048])
            nc.scalar.dma_start(out=wk_bf[:, 0:1024], in_=wk[:, 0:1024])
            xc_dma(nc.sync, 0, 0, 4)
            xc_dma(nc.gpsimd, 0, 4, 8)
            nc.scalar.dma_start(out=wk_bf[:, 1024:2048], in_=wk[:, 1024:2048])
            nc.scalar.dma_start(out=wv_bf[:], in_=wv[:])
            xc_dma(nc.sync, 1, 0, 4)
            xc_dma(nc.gpsimd, 1, 4, 8)
            nc.scalar.dma_start(out=wo_bf[:], in_=wo[:])
            xc_dma(nc.sync, 2, 0, 4)
            xc_dma(nc.gpsimd, 2, 4, 8)
            xc_dma(nc.sync, 3, 0, 4)
            xc_dma(nc.gpsimd, 3, 4, 8)

            nc.gpsimd.memset(ones_b[:], 1.0)
            nc.gpsimd.memset(junk[:], 0.0)
            # exp table prefetch: overlaps the ~2.7us ACT_TABLE_LOAD with
            # the input DMAs instead of paying it at the first real score
            nc.scalar.activation(out=jout[:], in_=junk[:], func=AF.Exp, scale=1.0)
            vb3 = vbuf.rearrange("p (t c) -> p t c", c=65)
            nc.gpsimd.memset(vb3[:, :, 64:65], 1.0)
            vb4 = vbuf.rearrange("p (n c) -> p n c", c=65)

            # ---- projection wavefront thunks (PE filler) ----
            def qk_thunks(c):
                th = []
                for w_bf, outT in ((wq_bf, QT), (wk_bf, KT)):
                    for m in range(2):
                        def go(w_bf=w_bf, outT=outT, m=m, c=c):
                            ps = ps_fill.tile([128, 512], F32, name="qk", tag="fill")
                            for dt_ in range(DT):
                                nc.tensor.matmul(
                                    ps[:],
                                    lhsT=w_bf[
                                        :,
                                        m * 1024 + dt_ * 128 : m * 1024 + (dt_ + 1) * 128,
                                    ],
                                    rhs=xT[
                                        :, dt_ * T + c * 512 : dt_ * T + (c + 1) * 512
                                    ],
                                    start=(dt_ == 0),
                                    stop=(dt_ == DT - 1),
                                )
                            nc.vector.tensor_copy(
                                outT[:, m * T + c * 512 : m * T + (c + 1) * 512],
                                ps[:],
                            )

                        th.append(go)
                return th

            def v_thunks(tts):
                th = []
                for tt in tts:
                    def go(tt=tt):
                        ps = ps_fill.tile([128, 256], F32, name="vp", tag="fill")
                        for dt_ in range(DT):
                            nc.tensor.matmul(
                                ps[:],
                                lhsT=xT[
                                    :, dt_ * T + tt * 128 : dt_ * T + (tt + 1) * 128
                                ],
                                rhs=wv_bf[:, dt_ * DH : (dt_ + 1) * DH],
                                start=(dt_ == 0),
                                stop=(dt_ == DT - 1),
                            )
                        nc.vector.tensor_copy(
                            vb4[:, tt * NH : (tt + 1) * NH, 0:64],
                            ps.rearrange("p (n c) -> p n c", n=NH),
                        )

                    th.append(go)
                return th

            # ---- scores ----
            def pt_layout(s):
                """Compact per-pair PT layout: col base and q-offset per kt."""
                base, off, b = {}, {}, 0
                for kt in range(4 * (s + 1)):
                    j = kt - 4 * s
                    o = 128 * j if j > 0 else 0
                    base[kt], off[kt] = b, o
                    b += 512 - o
                return base, off

            def scores_chunks(s, h, pt):
                m, r0 = h // 2, (h % 2) * 64
                base, _ = pt_layout(s)

                def off_diag(kt):
                    def go():
                        ps = ps_big.tile([128, 1024], F32, name="psst")
                        for u in range(2):
                            nc.tensor.matmul(
                                ps[:, u * 512 : (u + 1) * 512],
                                lhsT=KT[
                                    r0 : r0 + 64,
                                    m * T + (kt + u) * 128 : m * T + (kt + u + 1) * 128,
                                ],
                                rhs=QT[
                                    r0 : r0 + 64,
                                    m * T + s * 512 : m * T + (s + 1) * 512,
                                ],
                                start=True,
                                stop=True,
                            )
                        nc.scalar.activation(
                            out=pt[:, base[kt] : base[kt] + 1024],
                            in_=ps[:],
                            func=AF.Exp,
                            scale=float(SCALE),
                        )

                    return go

                def diag2(j0):
                    widths = [512 - 128 * j0, 512 - 128 * (j0 + 1)]
                    wtot = sum(widths)

                    def go():
                        ps = ps_big.tile([128, 1024], F32, name="psst")
                        o = 0
                        for u, w in enumerate(widths):
                            j = j0 + u
                            kt = 4 * s + j
                            nc.tensor.matmul(
                                ps[:, o : o + w],
                                lhsT=KT[
                                    r0 : r0 + 64,
                                    m * T + kt * 128 : m * T + (kt + 1) * 128,
                                ],
                                rhs=QT[
                                    r0 : r0 + 64,
                                    m * T + s * 512 + 128 * j : m * T + (s + 1) * 512,
                                ],
                                start=True,
                                stop=True,
                            )
                            o += w
                        kt0 = 4 * s + j0
                        nc.scalar.activation(
                            out=pt[:, base[kt0] : base[kt0] + wtot],
                            in_=ps[:, 0:wtot],
                            func=AF.Exp,
                            scale=float(SCALE),
                        )
                        for u in range(2):
                            kt = 4 * s + j0 + u
                            nc.gpsimd.affine_select(
                                out=pt[:, base[kt] : base[kt] + 128],
                                in_=pt[:, base[kt] : base[kt] + 128],
                                pattern=[[1, 128]],
                                compare_op=mybir.AluOpType.is_ge,
                                fill=0.0,
                                base=0,
                                channel_multiplier=-1,
                            )

                    return go

                return [off_diag(2 * u) for u in range(2 * s)] + [diag2(0), diag2(2)]

            # ---- AV + normalize ----
            def av_ops_ab(s, h, pt, ref):
                """AV accumulation split: A = off-diag k-tiles (exp long
                done), B = the 4 diagonal k-tiles (gated on last exps)."""
                base, off = pt_layout(s)
                nk = 4 * (s + 1)

                def av_a():
                    avb = ps_av.tile([128, 512], F32, name="psav", tag="psav")
                    ref["avb"] = avb
                    for kt in range(4 * s):
                        nc.tensor.matmul(
                            avb[0:65, 0:512],
                            lhsT=vb4[:, kt * NH + h, :],
                            rhs=pt[:, base[kt] : base[kt] + 512],
                            start=(kt == 0),
                            stop=False,
                        )

                def av_b():
                    avb = ref["avb"]
                    for kt in range(4 * s, nk):
                        o = off[kt]
                        nc.tensor.matmul(
                            avb[0:65, o:512],
                            lhsT=vb4[:, kt * NH + h, :],
                            rhs=pt[:, base[kt] : base[kt] + 512 - o],
                            start=(kt == 0),
                            stop=(kt == nk - 1),
                        )

                return av_a, av_b

            def norm_ops(s, h, ref, q0, q1):
                """den row -> bf16 -> K=1 ones-matmul broadcast -> recip
                -> tensor_tensor multiply -> attnT (DMA shift for odd h)."""
                i_c = h // 2
                c0 = i_c * T + s * 512
                odd = h % 2 == 1
                st = {}

                def d1():
                    den = den_pool.tile([128, 512], BF16, name="den")
                    st["den"] = den
                    nc.vector.tensor_copy(
                        den[64:65, q0:q1], ref["avb"][64:65, q0:q1]
                    )

                def m1():
                    denb = ps_fill.tile([128, 512], F32, name="denb", tag="fill")
                    st["denb"] = denb
                    nc.tensor.matmul(
                        denb[0:64, q0:q1],
                        lhsT=ones_b[64:65, 0:64],
                        rhs=st["den"][64:65, q0:q1],
                        start=True,
                        stop=True,
                    )

                def d2():
                    rc = rc_pool.tile([128, 512], F32, name="rc")
                    st["rc"] = rc
                    nc.vector.reciprocal_approx_fast(
                        rc[0:64, q0:q1], st["denb"][0:64, q0:q1]
                    )

                def d3():
                    if odd:
                        stg = stg_pool.tile([128, 512], BF16, name="stg")
                        st["stg"] = stg
                        dst = stg[0:64, q0:q1]
                    else:
                        dst = attnT[0:64, c0 + q0 : c0 + q1]
                    nc.vector.tensor_tensor(
                        out=dst,
                        in0=ref["avb"][0:64, q0:q1],
                        in1=st["rc"][0:64, q0:q1],
                        op=MULT,
                    )

                def d4():
                    nc.gpsimd.dma_start(
                        out=attnT[64:128, c0 + q0 : c0 + q1],
                        in_=st["stg"][0:64, q0:q1],
                    )

                ops = [d1, m1, d2, d3]
                if odd:
                    ops.append(d4)
                return ops

            # ---- out projection epilogue ----
            def epilogue_ops(s):
                ops = []
                for tt in range(4 * s, 4 * (s + 1)):
                    st = {}

                    def op_ec(ec, tt=tt, st=st):
                        def go():
                            ps = ps_fill.tile([128, 512], F32, name="opj", tag="fill")
                            st[ec] = ps
                            for i in range(2):
                                nc.tensor.matmul(
                                    ps[:],
                                    lhsT=attnT[
                                        :, i * T + tt * 128 : i * T + (tt + 1) * 128
                                    ],
                                    rhs=wo_bf[
                                        :, i * D + ec * 512 : i * D + (ec + 1) * 512
                                    ],
                                    start=(i == 0),
                                    stop=(i == 1),
                                )

                        return go

                    def cast_dma(ec, tt=tt, st=st):
                        def go():
                            if ec == 0:
                                st["osb"] = osb_pool.tile(
                                    [128, 1024], BF16, name="osb"
                                )
                            nc.vector.tensor_copy(
                                st["osb"][:, ec * 512 : (ec + 1) * 512], st[ec][:]
                            )
                            if ec == 1:
                                eng = nc.sync if tt % 2 == 0 else nc.gpsimd
                                eng.dma_start(
                                    out=out[tt * 128 : (tt + 1) * 128, :],
                                    in_=st["osb"][:],
                                )

                        return go

                    ops.extend([op_ec(0), op_ec(1), cast_dma(0), cast_dma(1)])
                return ops

            def interleave(a, b):
                if not a:
                    return list(b)
                if not b:
                    return list(a)
                res = []
                nb, na, bi = len(b), len(a), 0
                for i, op in enumerate(a):
                    res.append(op)
                    want = (i + 1) * nb // na
                    while bi < want:
                        res.append(b[bi])
                        bi += 1
                res.extend(b[bi:])
                return res

            # ---- prologue: wave 0 (QK chunk 0 + V tiles 0-3) ----
            for op in qk_thunks(0):
                op()
            for op in v_thunks(range(0, 4)):
                op()

            waves = {
                0: qk_thunks(1) + v_thunks(range(4, 8)),
                1: qk_thunks(2) + v_thunks(range(8, 12)),
                2: qk_thunks(3) + v_thunks(range(12, 16)),
            }

            HORD = [1, 3, 0, 2]  # odd heads first: their attnT DMA-shifts
            # happen early; the final pair of each slab writes attnT direct
            pairs = [(s, h) for s in range(NSLAB) for h in HORD]
            pts = {}
            refs = {}
            prev = None
            pending_epi = []
            for idx in range(len(pairs)):
                s, h = pairs[idx]
                pts[idx] = pt_pool.tile([128, TT * 512], BF16, name="pt")
                sc = scores_chunks(s, h, pts[idx])
                blist = []
                if prev is not None:
                    ps_, ph_ = pairs[prev]
                    refs[prev] = {}
                    av_a, av_b = av_ops_ab(ps_, ph_, pts[prev], refs[prev])
                    nrm = norm_ops(ps_, ph_, refs[prev], 0, 512)
                    epi = list(pending_epi)
                    pending_epi = []
                    if idx % 4 == 1 and s >= 1:
                        eall = epilogue_ops(s - 1)
                        epi += eall[:8]
                        pending_epi = eall[8:]
                    fill = waves[s][2 * (idx % 4) : 2 * (idx % 4) + 2] if s < 3 else []
                    blist = (
                        [av_a]
                        + epi[:6]
                        + fill
                        + [av_b]
                        + nrm
                        + epi[6:]
                    )
                else:
                    fill = waves[0][0:2]
                    blist = list(fill)
                for op in interleave(sc, blist):
                    op()
                prev = idx
            assert not pending_epi

            # ---- final pair (3,3): split AV/norm halves interleaved with
            # the slab-3 epilogue ----
            s_, h_ = 3, 2
            ref = {}
            base, off = pt_layout(s_)
            nk = 16
            pt = pts[15]

            def av15a():
                avb = ps_av.tile([128, 512], F32, name="psav", tag="psav")
                ref["avb"] = avb
                kts = [kt for kt in range(nk) if off[kt] < 256]
                for i_, kt in enumerate(kts):
                    o = off[kt]
                    nc.tensor.matmul(
                        avb[0:65, o:256],
                        lhsT=vb4[:, kt * NH + h_, :],
                        rhs=pt[:, base[kt] : base[kt] + 256 - o],
                        start=(i_ == 0),
                        stop=(i_ == len(kts) - 1),
                    )

            def av15b():
                avb = ref["avb"]
                for kt in range(nk):
                    o2 = max(off[kt] - 256, 0)
                    c0 = base[kt] + 256 - off[kt] + o2
                    nc.tensor.matmul(
                        avb[0:65, 256 + o2 : 512],
                        lhsT=vb4[:, kt * NH + h_, :],
                        rhs=pt[:, c0 : c0 + 256 - o2],
                        start=(kt == 0),
                        stop=(kt == nk - 1),
                    )

            nrm_a = norm_ops(s_, h_, ref, 0, 256)
            nrm_b = norm_ops(s_, h_, ref, 256, 512)
            eops = epilogue_ops(3)

            av15a()
            for op_ in nrm_a:
                op_()
            for op_ in eops[0:4]:  # tt12
                op_()
            av15b()
            for op_ in eops[4:8]:  # tt13
                op_()
            for op_ in nrm_b:
                op_()
            for op_ in eops[8:16]:  # tt14, tt15
                op_()

    nc.compile()
    return nc


def _get_nc():
    global _NC_CACHE
    if _NC_CACHE is None:
        _NC_CACHE = build()
    return _NC_CACHE


def make_in_maps(x, Wq, Wk, Wv, Wo):
    bf = ml_dtypes.bfloat16
    x = np.asarray(x, dtype=np.float32)
    WqT = np.asarray(Wq, dtype=np.float32).astype(bf)
    WkT = np.asarray(Wk, dtype=np.float32).astype(bf)
    WvT = np.asarray(Wv, dtype=np.float32).astype(bf)
    WoT = np.asarray(Wo, dtype=np.float32).astype(bf)

    def x_img(xb):  # [1024(d), 2048(t)] -> [128, c*4096 + dt*512 + j]
        return np.ascontiguousarray(
            xb.reshape(DT, 128, NSLAB, 512).transpose(1, 2, 0, 3).reshape(128, -1)
        )

    def qk_img(w):  # [1024, 256] -> m-major [128, m*1024 + dt*128 + c]
        return np.ascontiguousarray(
            w.reshape(DT, 128, 2, 128).transpose(1, 2, 0, 3).reshape(128, -1)
        )

    def v_img(w):  # [1024, 256] -> dt-major [128, dt*256 + c]
        return np.ascontiguousarray(
            w.reshape(DT, 128, DH).transpose(1, 0, 2).reshape(128, -1)
        )

    def o_img(w):  # [256, 1024] -> i-major [128, i*1024 + c]
        return np.ascontiguousarray(
            w.reshape(2, 128, D).transpose(1, 0, 2).reshape(128, -1)
        )

    xTb = [x_img(x[b].T.astype(bf)) for b in range(2)]
    in_maps = []
    for core in range(8):
        b, g = core // 4, core % 4
        sl = slice(g * DH, (g + 1) * DH)
        in_maps.append(
            {
                "xT": xTb[b],
                "Wq": qk_img(WqT[:, sl]),
                "Wk": qk_img(WkT[:, sl]),
                "Wv": v_img(WvT[:, sl]),
                "Wo": o_img(WoT[sl, :]),
            }
        )
    return in_maps


def unshard(results):
    out = np.empty((2, T, D), np.float32)
    for b in range(2):
        acc = results[4 * b]["out"].astype(np.float32)
        for g in range(1, 4):
            acc = acc + results[4 * b + g]["out"].astype(np.float32)
        out[b] = acc
    return out


def kernel(x, Wq, Wk, Wv, Wo):
    nc = _get_nc()
    in_maps = make_in_maps(x, Wq, Wk, Wv, Wo)
    res = run_bass_kernel_spmd(nc, in_maps, core_ids=list(range(8)))
    return unshard(res.results)
